# revision 16
# baseline (speedup 1.0000x reference)
"""Multi-head causal self-attention (B=4, S=2048, D=1024, H=16) on 8 trn2 cores.

Sharding: core c = (batch b = c//2, head-group g = c%2 of 8 heads).
Each core computes Q/K/V projections for its 8 heads over its batch's 2048
tokens, causal attention, and a PARTIAL output projection over its 512
feature dims. The host adds the two partial [2048, 1024] outputs per batch.
No on-device collectives.

All matmul operands are bf16 (error ~3e-3 vs the 2e-2 gate); PSUM stays
f32. Design is driven by measured per-instruction HW costs (~90-270ns fixed
per matmul, K=64 matmuls 1.6x slower than K=128):
  - Projections: single-pass bf16, 8-matmul accumulation chains (K=128).
  - Scores: one K=128 matmul per (head, s-tile): K is zero-padded by
    keeping per-head Q tiles (qtA/qtB) with the OTHER head's 64 rows
    pinned to zero, so no K=64 tile-mode penalty and no extra matmuls.
  - The causal mask costs zero PE work: score tiles are trimmed to the
    pair-level causal window and the sub-diagonal triangle of E is zeroed
    by gpsimd affine_select on the bf16 exp output.
  - A ones-column in V makes each PV matmul also emit the softmax
    denominator Z; PV accumulates in bf16 (full-rate accumulation).
  - W_O runs bf16 on [att * (1/Z)] tiles.
  - Projections(j), W_O(j-1), attention(j) are emitted in one loop so the
    PE always has ready work at slice boundaries and ACT exp overlaps PE.
Constant tiles (q zero-halves, V ones column, exp bias) live outside the
rep loop.
"""

import ml_dtypes
import numpy as np

import concourse.bass as bass
import concourse.mybir as mybir
import concourse.tile as tile
from concourse import bacc
from concourse.bass_utils import run_bass_kernel_spmd

F32 = mybir.dt.float32
BF16 = mybir.dt.bfloat16
AF = mybir.ActivationFunctionType
NPBF = ml_dtypes.bfloat16

B = 4
S = 2048
D_MODEL = 1024
E = 512          # feature dims per core (8 heads x 64)
HEADS = 8
DK = 64
NQ = 4           # 512-token q-slices
ND = 8           # 128-dim d_model tiles
NT = 16          # 128-token tiles
SCALE = 0.125
EBIAS = -1.0     # exp headroom shift; cancels in softmax ratio


def _persist(tc, pool):
    nc = tc.nc
    t = {}
    # Q tiles with the other head's rows pinned to zero (K=128 scores)
    t["qtA"] = [pool.tile([128, S], BF16, name=f"qtA{hp}") for hp in range(4)]
    t["qtB"] = [pool.tile([128, S], BF16, name=f"qtB{hp}") for hp in range(4)]
    t["kt"] = [pool.tile([128, S], BF16, name=f"kt{hp}") for hp in range(4)]
    t["vt"] = pool.tile([128, NT, HEADS, DK + 1], BF16, name="vt")
    t["ebias"] = pool.tile([128, 1], F32, name="ebias")
    for hp in range(4):
        nc.gpsimd.memset(t["qtA"][hp][64:128, :], 0.0)
        nc.gpsimd.memset(t["qtB"][hp][0:64, :], 0.0)
    nc.gpsimd.memset(t["vt"][:, :, :, DK:DK + 1], 1.0)
    nc.gpsimd.memset(t["ebias"], EBIAS)
    return t


def _emit(tc, t, xT, wqT, wkT, wvT, woT, out):
    nc = tc.nc
    qtA, qtB, kt = t["qtA"], t["qtB"], t["kt"]
    vt, ebias = t["vt"], t["ebias"]
    with (
        tc.tile_pool(name="wpool", bufs=2) as wpool,
        tc.tile_pool(name="xp", bufs=16) as xp,
        tc.tile_pool(name="stp", bufs=2, space="PSUM") as stp,
        tc.tile_pool(name="pvp", bufs=4, space="PSUM") as pvp,
        tc.tile_pool(name="exq", bufs=6) as exq,
        tc.tile_pool(name="attp", bufs=8) as attp,
        tc.tile_pool(name="nrmp", bufs=6) as nrmp,
        tc.tile_pool(name="outp", bufs=4) as outp,
    ):
        wq = wpool.tile([128, ND, E], BF16, tag="wq", name="wq")
        wk = wpool.tile([128, ND, E], BF16, tag="wk", name="wk")
        wv = wpool.tile([128, ND, E], BF16, tag="wv", name="wv")
        wot = wpool.tile([128, 4, D_MODEL], BF16, tag="wot", name="wot")
        for tt, dd in ((wq, wqT), (wk, wkT), (wv, wvT), (wot, woT)):
            nc.sync.dma_start(out=tt, in_=dd)

        def emit_wo(jw, att_w):
            for tt in range(4):
                ot = outp.tile([128, D_MODEL], F32, tag="ot", name=f"ot{jw}{tt}")
                ttsl = slice(tt * 128, (tt + 1) * 128)
                for eo in range(2):
                    pso = stp.tile([128, 512], F32, tag="st",
                                   name=f"po{jw}{tt}{eo}")
                    for f in range(4):
                        nc.tensor.matmul(pso, att_w[f][:, ttsl],
                                         wot[:, f, eo * 512:(eo + 1) * 512],
                                         start=(f == 0), stop=(f == 3))
                    nc.vector.tensor_copy(ot[:, eo * 512:(eo + 1) * 512], pso)
                t0 = jw * 512 + tt * 128
                nc.sync.dma_start(out=out[t0:t0 + 128, :], in_=ot)

        prev_att = None
        for j in range(NQ):
            tsl = slice(j * 512, (j + 1) * 512)
            xc = []
            for d in range(ND):
                ck = xp.tile([128, 512], BF16, tag="x", name=f"x{j}_{d}")
                nc.sync.dma_start(out=ck, in_=xT[j, d])
                xc.append(ck)

            # ---- Q/K projections (bf16, 8-chains) ----
            for w8, dst in ((wq, "q"), (wk, kt)):
                for es in range(4):
                    esl = slice(es * 128, (es + 1) * 128)
                    ps = stp.tile([128, 512], F32, tag="st", name=f"qk{j}{es}")
                    for d in range(ND):
                        nc.tensor.matmul(ps, w8[:, d, esl], xc[d],
                                         start=(d == 0), stop=(d == ND - 1))
                    if dst == "q":
                        nc.vector.tensor_copy(qtA[es][0:64, tsl], ps[0:64, :])
                        nc.vector.tensor_copy(qtB[es][64:128, tsl],
                                              ps[64:128, :])
                    else:
                        nc.vector.tensor_copy(dst[es][:, tsl], ps)

            # ---- V projection (bf16, 8-chains) ----
            for u in range(4):
                usl = slice(u * 128, (u + 1) * 128)
                ps = stp.tile([128, E], F32, tag="st", name=f"v{j}_{u}")
                for d in range(ND):
                    nc.tensor.matmul(ps, xc[d][:, usl], wv[:, d, :],
                                     start=(d == 0), stop=(d == ND - 1))
                nc.vector.tensor_copy(
                    vt[:, 4 * j + u, :, 0:DK],
                    ps.rearrange("p (h k) -> p h k", h=HEADS))

            if prev_att is not None:
                emit_wo(j - 1, prev_att)

            # ---- attention for q-slice j ----
            att_j = [attp.tile([128, 512], BF16, tag="at",
                               name=f"at{j}_{f}") for f in range(4)]
            npair = 2 * (j + 1)
            for hp in range(4):
                hA, hB = 2 * hp, 2 * hp + 1
                pvA = pvp.tile([DK + 1, 512], F32, tag="pv", name=f"pvA{j}{hp}")
                pvB = pvp.tile([DK + 1, 512], F32, tag="pv", name=f"pvB{j}{hp}")
                def emit_pv(g, q0, exA, exB):
                    for u in range(2):
                        i = 2 * g + u
                        nc.tensor.matmul(pvA[:, q0:], vt[:, i, hA, :],
                                         exA[:, u, q0:],
                                         start=(g == 0 and u == 0),
                                         stop=(g == npair - 1 and u == 1))
                        nc.tensor.matmul(pvB[:, q0:], vt[:, i, hB, :],
                                         exB[:, u, q0:],
                                         start=(g == 0 and u == 0),
                                         stop=(g == npair - 1 and u == 1))

                pend = None
                for g in range(npair):
                    r0 = 2 * g - 4 * j
                    q0 = 128 * r0 if r0 > 0 else 0
                    qv = slice(j * 512 + q0, (j + 1) * 512)
                    stA = stp.tile([128, 2, 512], F32, tag="st",
                                   name=f"stA{j}{hp}{g}")
                    stB = stp.tile([128, 2, 512], F32, tag="st",
                                   name=f"stB{j}{hp}{g}")
                    exA = exq.tile([128, 2, 512], BF16, tag="ex",
                                   name=f"exA{j}{hp}{g}")
                    exB = exq.tile([128, 2, 512], BF16, tag="ex",
                                   name=f"exB{j}{hp}{g}")
                    for u in range(2):
                        i = 2 * g + u
                        ssl = slice(i * 128, (i + 1) * 128)
                        nc.tensor.matmul(stA[:, u, q0:], kt[hp][:, ssl],
                                         qtA[hp][:, qv], start=True, stop=True)
                        nc.tensor.matmul(stB[:, u, q0:], kt[hp][:, ssl],
                                         qtB[hp][:, qv], start=True, stop=True)
                    nc.scalar.activation(exA[:, :, q0:], stA[:, :, q0:],
                                         AF.Exp, scale=SCALE, bias=ebias)
                    nc.scalar.activation(exB[:, :, q0:], stB[:, :, q0:],
                                         AF.Exp, scale=SCALE, bias=ebias)
                    if r0 >= 0:
                        # zero E where s > q. Only the first 256 columns of
                        # the window can violate causality (u=0 diagonal
                        # block, u=1 invalid+diagonal blocks); beyond
                        # q0+256, q >= p + 128*(r0+u) always holds.
                        w = min(256, 512 - q0)
                        for ex in (exA, exB):
                            nc.gpsimd.affine_select(
                                out=ex[:, :, q0:q0 + w],
                                in_=ex[:, :, q0:q0 + w],
                                pattern=[[-128, 2], [1, w]],
                                compare_op=mybir.AluOpType.is_ge,
                                fill=0.0, base=0, channel_multiplier=-1)
                    # software-pipeline: PV of the previous pair goes out
                    # AFTER this pair's scores so the PE never waits on exp
                    if pend is not None:
                        emit_pv(*pend)
                    pend = (g, q0, exA, exB)
                emit_pv(*pend)
                for pvx, h in ((pvA, hA), (pvB, hB)):
                    rz = nrmp.tile([1, 512], F32, tag="rz", name=f"rz{j}{h}")
                    bz = nrmp.tile([DK, 512], F32, tag="bz", name=f"bz{j}{h}")
                    nc.vector.reciprocal(rz, pvx[DK:DK + 1, :])
                    nc.gpsimd.partition_broadcast(bz, rz)
                    f, ho = divmod(h, 2)
                    p0 = 64 * ho
                    nc.vector.tensor_mul(att_j[f][p0:p0 + 64, :],
                                         pvx[0:DK, :], bz)

            prev_att = att_j
        emit_wo(NQ - 1, prev_att)


def build_nc(reps=1):
    nc = bacc.Bacc(None, target_bir_lowering=False, debug=False)
    xT = nc.dram_tensor("xT", [NQ, ND, 128, 512], BF16, kind="ExternalInput")
    wqT = nc.dram_tensor("wqT", [128, ND, E], BF16, kind="ExternalInput")
    wkT = nc.dram_tensor("wkT", [128, ND, E], BF16, kind="ExternalInput")
    wvT = nc.dram_tensor("wvT", [128, ND, E], BF16, kind="ExternalInput")
    woT = nc.dram_tensor("woT", [128, 4, D_MODEL], BF16, kind="ExternalInput")
    out = nc.dram_tensor("out", [S, D_MODEL], F32, kind="ExternalOutput")
    aps = (xT.ap(), wqT.ap(), wkT.ap(), wvT.ap(), woT.ap(), out.ap())
    with tile.TileContext(nc) as tc:
        with tc.tile_pool(name="persist", bufs=1) as pool:
            t = _persist(tc, pool)
            if reps == 1:
                _emit(tc, t, *aps)
            else:
                with tc.For_i(0, reps, 1):
                    _emit(tc, t, *aps)
    nc.compile()
    return nc


def make_in_maps(x, W_Q, W_K, W_V, W_O):
    in_maps = []
    xT_b = []
    for b in range(B):
        xt = x[b].T.astype(NPBF)  # [D, S]
        xT_b.append(np.ascontiguousarray(
            xt.reshape(ND, 128, NQ, 512).transpose(2, 0, 1, 3)))
    for c in range(8):
        b, g = divmod(c, 2)
        sl = slice(g * E, (g + 1) * E)
        def wtile(w):  # [D, E] -> [128, ND, E]
            return np.ascontiguousarray(
                w.reshape(ND, 128, -1).transpose(1, 0, 2).astype(NPBF))
        in_maps.append({
            "xT": xT_b[b],
            "wqT": wtile(W_Q[sl, :].T),
            "wkT": wtile(W_K[sl, :].T),
            "wvT": wtile(W_V[sl, :].T),
            "woT": np.ascontiguousarray(
                W_O[:, sl].T.reshape(4, 128, D_MODEL)
                .transpose(1, 0, 2).astype(NPBF)),
        })
    return in_maps


_NC_CACHE = None


def kernel(x, W_Q, W_K, W_V, W_O, _trace=False):
    global _NC_CACHE
    if _NC_CACHE is None:
        _NC_CACHE = build_nc()
    nc = _NC_CACHE
    in_maps = make_in_maps(x, W_Q, W_K, W_V, W_O)
    r = run_bass_kernel_spmd(nc, in_maps, list(range(8)), trace=_trace)
    kernel.last_result = r
    out = np.empty((B, S, D_MODEL), np.float32)
    for b in range(B):
        out[b] = r.results[2 * b]["out"] + r.results[2 * b + 1]["out"]
    return out


# revision 17
# speedup vs baseline: 1.0940x; 1.0940x over previous
"""Multi-head causal self-attention (B=4, S=2048, D=1024, H=16) on 8 trn2 cores.

Sharding: core c = (batch b = c//2, head-group g = c%2 of 8 heads).
Each core computes Q/K/V projections for its 8 heads over its batch's 2048
tokens, causal attention, and a PARTIAL output projection over its 512
feature dims. The host adds the two partial [2048, 1024] outputs per batch.
No on-device collectives.

All matmul operands are bf16 (error ~3e-3 vs the 2e-2 gate); PSUM stays
f32. Design is driven by measured per-instruction HW costs (~90-270ns fixed
per matmul, K=64 matmuls 1.6x slower than K=128):
  - Projections: single-pass bf16, 8-matmul accumulation chains (K=128).
  - Scores: one K=128 matmul per (head, s-tile): K is zero-padded by
    keeping per-head Q tiles (qtA/qtB) with the OTHER head's 64 rows
    pinned to zero, so no K=64 tile-mode penalty and no extra matmuls.
  - The causal mask costs zero PE work: score tiles are trimmed to the
    pair-level causal window and the sub-diagonal triangle of E is zeroed
    by gpsimd affine_select on the bf16 exp output.
  - A ones-column in V makes each PV matmul also emit the softmax
    denominator Z; PV accumulates in bf16 (full-rate accumulation).
  - W_O runs bf16 on [att * (1/Z)] tiles.
  - Projections(j), W_O(j-1), attention(j) are emitted in one loop so the
    PE always has ready work at slice boundaries and ACT exp overlaps PE.
Constant tiles (q zero-halves, V ones column, exp bias) live outside the
rep loop.
"""

import ml_dtypes
import numpy as np

import concourse.bass as bass
import concourse.mybir as mybir
import concourse.tile as tile
from concourse import bacc
from concourse.bass_utils import run_bass_kernel_spmd

F32 = mybir.dt.float32
BF16 = mybir.dt.bfloat16
AF = mybir.ActivationFunctionType
NPBF = ml_dtypes.bfloat16

B = 4
S = 2048
D_MODEL = 1024
E = 512          # feature dims per core (8 heads x 64)
HEADS = 8
DK = 64
NQ = 4           # 512-token q-slices
ND = 8           # 128-dim d_model tiles
NT = 16          # 128-token tiles
SCALE = 0.125
EBIAS = -1.0     # exp headroom shift; cancels in softmax ratio


def _persist(tc, pool):
    nc = tc.nc
    t = {}
    # Q tiles with the other head's rows pinned to zero (K=128 scores)
    t["qtA"] = [pool.tile([128, S], BF16, name=f"qtA{hp}") for hp in range(4)]
    t["qtB"] = [pool.tile([128, S], BF16, name=f"qtB{hp}") for hp in range(4)]
    t["kt"] = [pool.tile([128, S], BF16, name=f"kt{hp}") for hp in range(4)]
    t["vt"] = pool.tile([128, NT, HEADS, DK + 1], BF16, name="vt")
    t["ebias"] = pool.tile([128, 1], F32, name="ebias")
    for hp in range(4):
        nc.gpsimd.memset(t["qtA"][hp][64:128, :], 0.0)
        nc.gpsimd.memset(t["qtB"][hp][0:64, :], 0.0)
    nc.gpsimd.memset(t["vt"][:, :, :, DK:DK + 1], 1.0)
    nc.gpsimd.memset(t["ebias"], EBIAS)
    return t


def _emit(tc, t, xT, wqT, wkT, wvT, woT, out):
    nc = tc.nc
    qtA, qtB, kt = t["qtA"], t["qtB"], t["kt"]
    vt, ebias = t["vt"], t["ebias"]
    with (
        tc.tile_pool(name="wpool", bufs=2) as wpool,
        tc.tile_pool(name="xp", bufs=16) as xp,
        tc.tile_pool(name="stp", bufs=2, space="PSUM") as stp,
        tc.tile_pool(name="pvp", bufs=4, space="PSUM") as pvp,
        tc.tile_pool(name="exq", bufs=8) as exq,
        tc.tile_pool(name="attp", bufs=8) as attp,
        tc.tile_pool(name="nrmp", bufs=4) as nrmp,
        tc.tile_pool(name="outp", bufs=3) as outp,
    ):
        wq = wpool.tile([128, ND, E], BF16, tag="wq", name="wq")
        wk = wpool.tile([128, ND, E], BF16, tag="wk", name="wk")
        wv = wpool.tile([128, ND, E], BF16, tag="wv", name="wv")
        wot = wpool.tile([128, 4, D_MODEL], BF16, tag="wot", name="wot")
        for tt, dd in ((wq, wqT), (wk, wkT), (wv, wvT), (wot, woT)):
            nc.sync.dma_start(out=tt, in_=dd)

        def emit_wo(jw, att_w):
            for tt in range(4):
                ot = outp.tile([128, D_MODEL], F32, tag="ot", name=f"ot{jw}{tt}")
                ttsl = slice(tt * 128, (tt + 1) * 128)
                for eo in range(2):
                    pso = stp.tile([128, 512], F32, tag="st",
                                   name=f"po{jw}{tt}{eo}")
                    for f in range(4):
                        nc.tensor.matmul(pso, att_w[f][:, ttsl],
                                         wot[:, f, eo * 512:(eo + 1) * 512],
                                         start=(f == 0), stop=(f == 3))
                    nc.vector.tensor_copy(ot[:, eo * 512:(eo + 1) * 512], pso)
                t0 = jw * 512 + tt * 128
                nc.sync.dma_start(out=out[t0:t0 + 128, :], in_=ot)

        prev_att = None
        for j in range(NQ):
            tsl = slice(j * 512, (j + 1) * 512)
            xc = []
            for d in range(ND):
                ck = xp.tile([128, 512], BF16, tag="x", name=f"x{j}_{d}")
                nc.sync.dma_start(out=ck, in_=xT[j, d])
                xc.append(ck)

            # ---- Q/K projections (bf16, 8-chains) ----
            for w8, dst in ((wq, "q"), (wk, kt)):
                for es in range(4):
                    esl = slice(es * 128, (es + 1) * 128)
                    ps = stp.tile([128, 512], F32, tag="st", name=f"qk{j}{es}")
                    for d in range(ND):
                        nc.tensor.matmul(ps, w8[:, d, esl], xc[d],
                                         start=(d == 0), stop=(d == ND - 1))
                    if dst == "q":
                        nc.vector.tensor_copy(qtA[es][0:64, tsl], ps[0:64, :])
                        nc.vector.tensor_copy(qtB[es][64:128, tsl],
                                              ps[64:128, :])
                    else:
                        nc.vector.tensor_copy(dst[es][:, tsl], ps)

            # ---- V projection (bf16, 8-chains) ----
            for u in range(4):
                usl = slice(u * 128, (u + 1) * 128)
                ps = stp.tile([128, E], F32, tag="st", name=f"v{j}_{u}")
                for d in range(ND):
                    nc.tensor.matmul(ps, xc[d][:, usl], wv[:, d, :],
                                     start=(d == 0), stop=(d == ND - 1))
                nc.vector.tensor_copy(
                    vt[:, 4 * j + u, :, 0:DK],
                    ps.rearrange("p (h k) -> p h k", h=HEADS))

            if prev_att is not None:
                emit_wo(j - 1, prev_att)

            # ---- attention for q-slice j ----
            att_j = [attp.tile([128, 512], BF16, tag="at",
                               name=f"at{j}_{f}") for f in range(4)]
            npair = 2 * (j + 1)
            for hp in range(4):
                hA, hB = 2 * hp, 2 * hp + 1
                pvA = pvp.tile([DK + 1, 512], F32, tag="pv", name=f"pvA{j}{hp}")
                pvB = pvp.tile([DK + 1, 512], F32, tag="pv", name=f"pvB{j}{hp}")
                def emit_pv(g, q0, exA, exB):
                    for u in range(2):
                        i = 2 * g + u
                        nc.tensor.matmul(pvA[:, q0:], vt[:, i, hA, :],
                                         exA[:, u, q0:],
                                         start=(g == 0 and u == 0),
                                         stop=(g == npair - 1 and u == 1))
                        nc.tensor.matmul(pvB[:, q0:], vt[:, i, hB, :],
                                         exB[:, u, q0:],
                                         start=(g == 0 and u == 0),
                                         stop=(g == npair - 1 and u == 1))

                pend = None
                for g in range(npair):
                    r0 = 2 * g - 4 * j
                    q0 = 128 * r0 if r0 > 0 else 0
                    qv = slice(j * 512 + q0, (j + 1) * 512)
                    stA = stp.tile([128, 2, 512], F32, tag="st",
                                   name=f"stA{j}{hp}{g}")
                    stB = stp.tile([128, 2, 512], F32, tag="st",
                                   name=f"stB{j}{hp}{g}")
                    exA = exq.tile([128, 2, 512], BF16, tag="ex",
                                   name=f"exA{j}{hp}{g}")
                    exB = exq.tile([128, 2, 512], BF16, tag="ex",
                                   name=f"exB{j}{hp}{g}")
                    for u in range(2):
                        i = 2 * g + u
                        ssl = slice(i * 128, (i + 1) * 128)
                        nc.tensor.matmul(stA[:, u, q0:], kt[hp][:, ssl],
                                         qtA[hp][:, qv], start=True, stop=True)
                        nc.tensor.matmul(stB[:, u, q0:], kt[hp][:, ssl],
                                         qtB[hp][:, qv], start=True, stop=True)
                    nc.scalar.activation(exA[:, :, q0:], stA[:, :, q0:],
                                         AF.Exp, scale=SCALE, bias=ebias)
                    nc.scalar.activation(exB[:, :, q0:], stB[:, :, q0:],
                                         AF.Exp, scale=SCALE, bias=ebias)
                    if r0 >= 0:
                        # zero E where s > q. Only the first 256 columns of
                        # the window can violate causality (u=0 diagonal
                        # block, u=1 invalid+diagonal blocks); beyond
                        # q0+256, q >= p + 128*(r0+u) always holds.
                        w = min(256, 512 - q0)
                        for ex in (exA, exB):
                            nc.gpsimd.affine_select(
                                out=ex[:, :, q0:q0 + w],
                                in_=ex[:, :, q0:q0 + w],
                                pattern=[[-128, 2], [1, w]],
                                compare_op=mybir.AluOpType.is_ge,
                                fill=0.0, base=0, channel_multiplier=-1)
                    # software-pipeline: PV of the previous pair goes out
                    # AFTER this pair's scores so the PE never waits on exp
                    if pend is not None:
                        emit_pv(*pend)
                    pend = (g, q0, exA, exB)
                emit_pv(*pend)
                for pvx, h in ((pvA, hA), (pvB, hB)):
                    rz = nrmp.tile([1, 512], F32, tag="rz", name=f"rz{j}{h}")
                    bz = nrmp.tile([DK, 512], F32, tag="bz", name=f"bz{j}{h}")
                    nc.vector.reciprocal(rz, pvx[DK:DK + 1, :])
                    nc.gpsimd.partition_broadcast(bz, rz)
                    f, ho = divmod(h, 2)
                    p0 = 64 * ho
                    nc.vector.tensor_mul(att_j[f][p0:p0 + 64, :],
                                         pvx[0:DK, :], bz)

            prev_att = att_j
        emit_wo(NQ - 1, prev_att)


def build_nc(reps=1):
    nc = bacc.Bacc(None, target_bir_lowering=False, debug=False)
    xT = nc.dram_tensor("xT", [NQ, ND, 128, 512], BF16, kind="ExternalInput")
    wqT = nc.dram_tensor("wqT", [128, ND, E], BF16, kind="ExternalInput")
    wkT = nc.dram_tensor("wkT", [128, ND, E], BF16, kind="ExternalInput")
    wvT = nc.dram_tensor("wvT", [128, ND, E], BF16, kind="ExternalInput")
    woT = nc.dram_tensor("woT", [128, 4, D_MODEL], BF16, kind="ExternalInput")
    out = nc.dram_tensor("out", [S, D_MODEL], F32, kind="ExternalOutput")
    aps = (xT.ap(), wqT.ap(), wkT.ap(), wvT.ap(), woT.ap(), out.ap())
    with tile.TileContext(nc) as tc:
        with tc.tile_pool(name="persist", bufs=1) as pool:
            t = _persist(tc, pool)
            if reps == 1:
                _emit(tc, t, *aps)
            else:
                with tc.For_i(0, reps, 1):
                    _emit(tc, t, *aps)
    nc.compile()
    return nc


def make_in_maps(x, W_Q, W_K, W_V, W_O):
    in_maps = []
    xT_b = []
    for b in range(B):
        xt = x[b].T.astype(NPBF)  # [D, S]
        xT_b.append(np.ascontiguousarray(
            xt.reshape(ND, 128, NQ, 512).transpose(2, 0, 1, 3)))
    for c in range(8):
        b, g = divmod(c, 2)
        sl = slice(g * E, (g + 1) * E)
        def wtile(w):  # [D, E] -> [128, ND, E]
            return np.ascontiguousarray(
                w.reshape(ND, 128, -1).transpose(1, 0, 2).astype(NPBF))
        in_maps.append({
            "xT": xT_b[b],
            "wqT": wtile(W_Q[sl, :].T),
            "wkT": wtile(W_K[sl, :].T),
            "wvT": wtile(W_V[sl, :].T),
            "woT": np.ascontiguousarray(
                W_O[:, sl].T.reshape(4, 128, D_MODEL)
                .transpose(1, 0, 2).astype(NPBF)),
        })
    return in_maps


_NC_CACHE = None


def kernel(x, W_Q, W_K, W_V, W_O, _trace=False):
    global _NC_CACHE
    if _NC_CACHE is None:
        _NC_CACHE = build_nc()
    nc = _NC_CACHE
    in_maps = make_in_maps(x, W_Q, W_K, W_V, W_O)
    r = run_bass_kernel_spmd(nc, in_maps, list(range(8)), trace=_trace)
    kernel.last_result = r
    out = np.empty((B, S, D_MODEL), np.float32)
    for b in range(B):
        out[b] = r.results[2 * b]["out"] + r.results[2 * b + 1]["out"]
    return out
